# revision 20
# baseline (speedup 1.0000x reference)
"""Trainium2 Bass kernel for masked grouped-bottleneck (moe_routing patch refine).

Full computation:
  x [16,1024,56,56] is split into a 7x7 grid of 8x8 patches; per patch a
  grouped (G=4) bottleneck conv1(1x1)->relu->conv2(3x3, per-patch pad)->relu
  ->conv3(1x1) runs; the result is zeroed for non-selected (b, group, patch)
  combos per `mask`, un-patchified, added to x (residual) and relu'd.

Sharding: data-parallel over batch, 2 images per core across 8 cores.
Weights are repacked on the host into PE-friendly lhsT layouts (block-diagonal
over group pairs so conv2 runs dense K=128/M=128 matmuls). The routing mask is
applied right after conv1: every conv is patch-local and bias-free, so zeroing
mid1 for a (group, patch) is exactly equivalent to zeroing the conv3 output.

All device-side data is bf16 (inputs cast on host, output up-cast on host):
that halves both load and store HBM traffic, leaving the PE as the bottleneck.
The relu+mask after conv1 is a single fused scalar_tensor_tensor
((p1 max 0) * mask); m1's zero patch borders are pre-set once; per-(b,pair)
output rows accumulate in SBUF so each 128-channel slab stores with one DMA.
"""
import numpy as np
import ml_dtypes

_CACHE = {}

B, C, H, W = 16, 1024, 56, 56
G, MS, HP = 4, 7, 8
MID = 256
NCORES = 8
BPC = B // NCORES   # batches per core

_BF16 = ml_dtypes.bfloat16


def _pack_weights(w1, w2, w3):
    w1s = np.zeros((128, 2 * 4 * 128), np.float32)
    for pair in range(2):
        for j in range(4):            # K-tile over the pair's 512 input chans
            gi, kt = j // 2, j % 2
            g = 2 * pair + gi
            Wg = w1[64 * g:64 * g + 64, 128 * kt:128 * kt + 128, 0, 0]
            w1s[:, (pair * 4 + j) * 128 + 64 * gi:(pair * 4 + j) * 128 + 64 * gi + 64] = Wg.T
    w2s = np.zeros((128, 2 * 9 * 128), np.float32)
    for pair in range(2):
        for tap in range(9):
            dy, dx = tap // 3, tap % 3
            for gi in range(2):
                g = 2 * pair + gi
                Wg = w2[64 * g:64 * g + 64, :, dy, dx]
                w2s[64 * gi:64 * gi + 64,
                    (pair * 9 + tap) * 128 + 64 * gi:(pair * 9 + tap) * 128 + 64 * gi + 64] = Wg.T
    w3s = np.zeros((128, 8 * 128), np.float32)
    for pair in range(2):
        for gi in range(2):
            g = 2 * pair + gi
            for mt in range(2):
                Wg = w3[256 * g + 128 * mt:256 * g + 128 * (mt + 1), :, 0, 0]
                blk = (pair * 2 + gi) * 2 + mt
                w3s[64 * gi:64 * gi + 64, blk * 128:(blk + 1) * 128] = Wg.T
    return w1s, w2s, w3s


def _pack_mask(mask_b):
    # mask_b: [BPC, 4, 7, 7] -> [128, BPC*2*7*56]: per (b, pair, patch-row py)
    # a 56-wide segment holding mask[(i % 7)] at col i -- one value per
    # (y, patch) pair of the row, so the fused relu+mask reads it with a
    # 3D broadcast AP. Partition halves hold the pair's two groups.
    m = np.zeros((128, BPC * 2 * 7 * 56), np.float32)
    mb = (mask_b > 0).astype(np.float32).reshape(BPC, 4, 7, 7)
    for b in range(BPC):
        for pair in range(2):
            for py in range(7):
                s = ((b * 2 + pair) * 7 + py) * 56
                seg = slice(s, s + 56)
                m[0:64, seg] = np.tile(mb[b, 2 * pair, py], 8)
                m[64:128, seg] = np.tile(mb[b, 2 * pair + 1, py], 8)
    return m


def _build_program(reps=1):
    import concourse.bacc as bacc
    import concourse.mybir as mybir
    import concourse.tile as tile

    f32 = mybir.dt.float32
    bf16 = mybir.dt.bfloat16
    Relu = mybir.ActivationFunctionType.Relu
    Amax = mybir.AluOpType.max
    Amul = mybir.AluOpType.mult

    nc = bacc.Bacc("TRN2", target_bir_lowering=False, debug=False)
    x_d = nc.dram_tensor("x", [BPC, C, H, W], bf16, kind="ExternalInput")
    mk_d = nc.dram_tensor("maskrep", [128, BPC * 2 * 7 * 56], f32, kind="ExternalInput")
    w1_d = nc.dram_tensor("w1s", [128, 1024], bf16, kind="ExternalInput")
    w2_d = nc.dram_tensor("w2s", [128, 2304], bf16, kind="ExternalInput")
    w3_d = nc.dram_tensor("w3s", [128, 1024], bf16, kind="ExternalInput")
    out_d = nc.dram_tensor("out", [BPC, C, H, W], bf16, kind="ExternalOutput")

    xap = x_d.ap()
    oap = out_d.ap()

    with tile.TileContext(nc) as tc:
        with (
            tc.tile_pool(name="wpool", bufs=1) as wpool,
            tc.tile_pool(name="xpool", bufs=3) as xpool,
            tc.tile_pool(name="m2pool", bufs=3) as m2pool,
            tc.tile_pool(name="t1pool", bufs=2) as t1pool,
            tc.tile_pool(name="t3pool", bufs=4) as t3pool,
            tc.tile_pool(name="opool", bufs=2) as opool,
            tc.tile_pool(name="ps1", bufs=2, space="PSUM") as ps1,
            tc.tile_pool(name="ps2", bufs=2, space="PSUM") as ps2,
            tc.tile_pool(name="ps3", bufs=4, space="PSUM") as ps3,
        ):
            w1t = wpool.tile([128, 1024], bf16, tag="w1")
            w2t = wpool.tile([128, 2304], bf16, tag="w2")
            w3t = wpool.tile([128, 1024], bf16, tag="w3")
            mkt = wpool.tile([128, BPC * 2 * 7 * 56], f32, tag="mk")
            # weights go on the ACT HWDGE ring so they don't queue ahead
            # of the first x-tile loads on the sync ring at startup
            nc.scalar.dma_start(w1t[:], w1_d.ap())
            nc.scalar.dma_start(w2t[:], w2_d.ap())
            nc.scalar.dma_start(w3t[:], w3_d.ap())
            nc.scalar.dma_start(mkt[:], mk_d.ap())

            # m1 (conv1 output, per-patch zero-padded 10x10) lives in 3
            # persistent buffers; the borders are zeroed once here and only
            # the inner 8x8 regions are rewritten each iteration.
            m1s = [wpool.tile([128, 700], bf16, tag=f"m1_{i}", name=f"m1_{i}")
                   for i in range(3)]
            for t in m1s:
                nc.gpsimd.memset(t[:], 0.0)

            # --- software-pipelined schedule ---------------------------------
            # Round k issues conv1(k) / conv2(k-1) / conv3(k-2) so the PE never
            # waits on the DVE mask-relu (feeds conv2) or the ACT m2-relu
            # (feeds conv3): each cross-engine producer gets a full round of
            # slack before its consumer.
            groups = []
            for rep in range(reps):
                for b in range(BPC):
                    for pair in range(2):
                        groups.append((b, pair))
            iters = [(b, pair, py) for (b, pair) in groups for py in range(MS)]
            n = len(iters)
            state = {}   # per-iteration/group tiles carried across stages

            def xrow(t, py):
                # a patch row in (y, px, x) order is just a contiguous
                # 448-column slab of the [c, h*w] tile
                return t[:, 448 * py:448 * (py + 1)]

            def load_group(g):
                b, pair = groups[g]
                xts = [xpool.tile([128, H * W], bf16, tag=f"x{i}",
                                  name=f"xt{g}_{i}") for i in range(4)]
                for i in range(4):
                    c0 = 512 * pair + 128 * i
                    nc.sync.dma_start(
                        xts[i][:],
                        xap[b, c0:c0 + 128].rearrange("c h w -> c (h w)"))
                oas = [opool.tile([128, H * W], bf16, tag=f"oa{i}",
                                  name=f"oa{g}_{i}") for i in range(4)]
                state[("grp", g)] = (xts, oas)

            def stage_a(k):
                b, pair, py = iters[k]
                # prefetch the next group's x tiles ~4.5 rounds before use
                if py == 2 and k // MS + 1 < len(groups):
                    load_group(k // MS + 1)
                xts, oas = state[("grp", k // MS)]
                # ---- conv1: 4 accumulating blockdiag matmuls ----
                p1 = ps1.tile([128, 448], f32, tag="p1", name=f"p1_{k}")
                for j in range(4):
                    nc.tensor.matmul(
                        p1[:],
                        w1t[:, (pair * 4 + j) * 128:(pair * 4 + j + 1) * 128],
                        xrow(xts[j], py),
                        start=(j == 0), stop=(j == 3))
                # ---- fused relu+mask (DVE): m1 = max(p1,0) * mask ----
                # m1 is [row10, patch7, col10] with zero borders; the inner
                # (row 1:9, patch, col 1:9) region collapses to a uniform
                # stride-10 dim of 56 x 8 contiguous cols, so out/in0/in1 are
                # all <=3D as TensorScalarPtr requires, and p1's (y, px, x)
                # stream order matches the (row, patch, col) write order.
                s0 = ((b * 2 + pair) * 7 + py) * 56
                mseg = mkt[:, s0:s0 + 56]
                mbc = mseg.unsqueeze(2).broadcast_to([128, 56, 8])
                m1 = m1s[k % 3]
                m1i = m1[:].rearrange("p (ra c) -> p ra c", c=10)[:, 7:63, 1:9]
                t1 = t1pool.tile([128, 448], bf16, tag="t1", name=f"t1_{k}")
                nc.scalar.activation(t1[:], p1[:], Relu)
                t1v = t1[:].rearrange("p (ra c) -> p ra c", c=8)
                nc.vector.tensor_tensor(m1i, t1v, mbc, Amul)

            def stage_b(k):
                b, pair, py = iters[k]
                m1 = m1s[k % 3]
                m1rc = m1[:].rearrange("p (ra c) -> p ra c", c=10)
                p2 = ps2.tile([128, 448], f32, tag="p2", name=f"p2_{k}")
                for tap in range(9):
                    dy, dx = tap // 3, tap % 3
                    rhs = m1rc[:, 7 * dy:7 * dy + 56, dx:dx + 8]
                    nc.tensor.matmul(
                        p2[:],
                        w2t[:, (pair * 9 + tap) * 128:(pair * 9 + tap + 1) * 128],
                        rhs,
                        start=(tap == 0), stop=(tap == 8))
                m2 = m2pool.tile([128, 448], bf16, tag="m2", name=f"m2_{k}")
                nc.scalar.activation(m2[:], p2[:], Relu)
                state[("m2", k)] = m2

            def stage_c(k):
                b, pair, py = iters[k]
                xts, oas = state[("grp", k // MS)]
                m2 = state.pop(("m2", k))
                for mt in range(2):
                    for gi in range(2):
                        blk = (pair * 2 + gi) * 2 + mt
                        p3 = ps3.tile([128, 448], f32, tag="p3", name=f"p3_{k}_{mt}_{gi}")
                        nc.tensor.matmul(
                            p3[:],
                            w3t[64 * gi:64 * gi + 64, blk * 128:(blk + 1) * 128],
                            m2[64 * gi:64 * gi + 64, :])
                        ct = 2 * gi + mt
                        t3 = t3pool.tile([128, 448], bf16, tag="t3", name=f"t3_{k}_{ct}")
                        # adds read PSUM -> must be DVE (Pool can't on HW, ACT
                        # can't add tensors); every AP here is a contiguous
                        # [128, 448] slab in the shared (y, px, x) order
                        nc.vector.tensor_add(t3[:], p3[:], xrow(xts[ct], py))
                        nc.scalar.activation(xrow(oas[ct], py), t3[:], Relu)
                if py == MS - 1:
                    for i in range(4):
                        c0 = 512 * pair + 128 * i
                        nc.sync.dma_start(
                            oap[b, c0:c0 + 128].rearrange("c h w -> c (h w)"),
                            oas[i][:])
                    del state[("grp", k // MS)]

            load_group(0)
            for k in range(n + 2):
                if k < n:
                    stage_a(k)
                if 1 <= k <= n:
                    stage_b(k - 1)
                if k >= 2:
                    stage_c(k - 2)
    nc.compile()
    return nc


def _get_program():
    if "nc" not in _CACHE:
        _CACHE["nc"] = _build_program()
    return _CACHE["nc"]


def make_in_maps(x, mask, w1, w2, w3):
    x = np.ascontiguousarray(np.asarray(x, np.float32)).astype(_BF16)
    mask = np.asarray(mask, np.float32)
    w1s, w2s, w3s = _pack_weights(np.asarray(w1, np.float32),
                                  np.asarray(w2, np.float32),
                                  np.asarray(w3, np.float32))
    w1s, w2s, w3s = w1s.astype(_BF16), w2s.astype(_BF16), w3s.astype(_BF16)
    in_maps = []
    for k in range(NCORES):
        in_maps.append({
            "x": x[BPC * k:BPC * (k + 1)],
            "maskrep": _pack_mask(mask[BPC * k:BPC * (k + 1)]),
            "w1s": w1s, "w2s": w2s, "w3s": w3s,
        })
    return in_maps


def kernel(x, mask, w1, w2, w3):
    from concourse import bass_utils

    in_maps = make_in_maps(x, mask, w1, w2, w3)
    nc = _get_program()
    res = bass_utils.run_bass_kernel_spmd(nc, in_maps, core_ids=list(range(NCORES)))
    out = np.concatenate([res.results[k]["out"] for k in range(NCORES)], axis=0)
    return out.astype(np.float32)


# revision 23
# speedup vs baseline: 1.0492x; 1.0492x over previous
"""Trainium2 Bass kernel for masked grouped-bottleneck (moe_routing patch refine).

Full computation:
  x [16,1024,56,56] is split into a 7x7 grid of 8x8 patches; per patch a
  grouped (G=4) bottleneck conv1(1x1)->relu->conv2(3x3, per-patch pad)->relu
  ->conv3(1x1) runs; the result is zeroed for non-selected (b, group, patch)
  combos per `mask`, un-patchified, added to x (residual) and relu'd.

Sharding: data-parallel over batch, 2 images per core across 8 cores.
Weights are repacked on the host into PE-friendly lhsT layouts (block-diagonal
over group pairs so conv2 runs dense K=128/M=128 matmuls). The routing mask is
applied right after conv1: every conv is patch-local and bias-free, so zeroing
mid1 for a (group, patch) is exactly equivalent to zeroing the conv3 output.

All device-side data is bf16 (inputs cast on host, output up-cast on host):
that halves both load and store HBM traffic, leaving the PE as the bottleneck.
The relu+mask after conv1 is a single fused scalar_tensor_tensor
((p1 max 0) * mask); m1's zero patch borders are pre-set once; per-(b,pair)
output rows accumulate in SBUF so each 128-channel slab stores with one DMA.
"""
import numpy as np
import ml_dtypes

_CACHE = {}

B, C, H, W = 16, 1024, 56, 56
G, MS, HP = 4, 7, 8
MID = 256
NCORES = 8
BPC = B // NCORES   # batches per core

_BF16 = ml_dtypes.bfloat16
_F8 = ml_dtypes.float8_e4m3


def _pack_weights(w1, w2, w3):
    w1s = np.zeros((128, 2 * 4 * 128), np.float32)
    for pair in range(2):
        for j in range(4):            # K-tile over the pair's 512 input chans
            gi, kt = j // 2, j % 2
            g = 2 * pair + gi
            Wg = w1[64 * g:64 * g + 64, 128 * kt:128 * kt + 128, 0, 0]
            w1s[:, (pair * 4 + j) * 128 + 64 * gi:(pair * 4 + j) * 128 + 64 * gi + 64] = Wg.T
    w2s = np.zeros((128, 2 * 9 * 128), np.float32)
    for pair in range(2):
        for tap in range(9):
            dy, dx = tap // 3, tap % 3
            for gi in range(2):
                g = 2 * pair + gi
                Wg = w2[64 * g:64 * g + 64, :, dy, dx]
                w2s[64 * gi:64 * gi + 64,
                    (pair * 9 + tap) * 128 + 64 * gi:(pair * 9 + tap) * 128 + 64 * gi + 64] = Wg.T
    w3s = np.zeros((128, 8 * 128), np.float32)
    for pair in range(2):
        for gi in range(2):
            g = 2 * pair + gi
            for mt in range(2):
                Wg = w3[256 * g + 128 * mt:256 * g + 128 * (mt + 1), :, 0, 0]
                blk = (pair * 2 + gi) * 2 + mt
                w3s[64 * gi:64 * gi + 64, blk * 128:(blk + 1) * 128] = Wg.T
    return w1s, w2s, w3s


def _pack_mask(mask_b):
    # mask_b: [BPC, 4, 7, 7] -> [128, BPC*2*7*56]: per (b, pair, patch-row py)
    # a 56-wide segment holding mask[(i % 7)] at col i -- one value per
    # (y, patch) pair of the row, so the fused relu+mask reads it with a
    # 3D broadcast AP. Partition halves hold the pair's two groups.
    m = np.zeros((128, BPC * 2 * 7 * 56), np.float32)
    mb = (mask_b > 0).astype(np.float32).reshape(BPC, 4, 7, 7)
    for b in range(BPC):
        for pair in range(2):
            for py in range(7):
                s = ((b * 2 + pair) * 7 + py) * 56
                seg = slice(s, s + 56)
                m[0:64, seg] = np.tile(mb[b, 2 * pair, py], 8)
                m[64:128, seg] = np.tile(mb[b, 2 * pair + 1, py], 8)
    return m


def _build_program(reps=1):
    import concourse.bacc as bacc
    import concourse.mybir as mybir
    import concourse.tile as tile

    f32 = mybir.dt.float32
    bf16 = mybir.dt.bfloat16
    f8 = mybir.dt.float8e4
    Relu = mybir.ActivationFunctionType.Relu
    Amax = mybir.AluOpType.max
    Amul = mybir.AluOpType.mult

    nc = bacc.Bacc("TRN2", target_bir_lowering=False, debug=False)
    x_d = nc.dram_tensor("x", [BPC, C, H, W], bf16, kind="ExternalInput")
    mk_d = nc.dram_tensor("maskrep", [128, BPC * 2 * 7 * 56], f32, kind="ExternalInput")
    w1_d = nc.dram_tensor("w1s", [128, 1024], bf16, kind="ExternalInput")
    w2_d = nc.dram_tensor("w2s", [128, 2304], f8, kind="ExternalInput")
    w3_d = nc.dram_tensor("w3s", [128, 1024], bf16, kind="ExternalInput")
    id_d = nc.dram_tensor("ident", [128, 128], bf16, kind="ExternalInput")
    out_d = nc.dram_tensor("out", [BPC, C, H, W], bf16, kind="ExternalOutput")

    xap = x_d.ap()
    oap = out_d.ap()

    with tile.TileContext(nc) as tc:
        with (
            tc.tile_pool(name="wpool", bufs=1) as wpool,
            tc.tile_pool(name="xpool", bufs=3) as xpool,
            tc.tile_pool(name="m2pool", bufs=3) as m2pool,
            tc.tile_pool(name="t1pool", bufs=2) as t1pool,
            tc.tile_pool(name="t3pool", bufs=4) as t3pool,
            tc.tile_pool(name="opool", bufs=2) as opool,
            tc.tile_pool(name="ps1", bufs=2, space="PSUM") as ps1,
            tc.tile_pool(name="ps2", bufs=2, space="PSUM") as ps2,
            tc.tile_pool(name="ps3", bufs=4, space="PSUM") as ps3,
        ):
            w1t = wpool.tile([128, 1024], bf16, tag="w1")
            w2t = wpool.tile([128, 2304], f8, tag="w2")
            w3t = wpool.tile([128, 1024], bf16, tag="w3")
            mkt = wpool.tile([128, BPC * 2 * 7 * 56], f32, tag="mk")
            idt = wpool.tile([128, 128], bf16, tag="ident")
            # weights go on the ACT HWDGE ring so they don't queue ahead
            # of the first x-tile loads on the sync ring at startup
            nc.scalar.dma_start(w1t[:], w1_d.ap())
            nc.scalar.dma_start(w2t[:], w2_d.ap())
            nc.scalar.dma_start(w3t[:], w3_d.ap())
            nc.scalar.dma_start(mkt[:], mk_d.ap())
            nc.scalar.dma_start(idt[:], id_d.ap())

            # m1 (conv1 output, per-patch zero-padded 10x10) lives in 3
            # persistent buffers; the borders are zeroed once here and only
            # the inner 8x8 regions are rewritten each iteration.
            m1s = [wpool.tile([128, 700], f8, tag=f"m1_{i}", name=f"m1_{i}")
                   for i in range(3)]
            for t in m1s:
                nc.gpsimd.memset(t[:], 0.0)

            # --- software-pipelined schedule ---------------------------------
            # Round k issues conv1(k) / conv2(k-1) / conv3(k-2) so the PE never
            # waits on the DVE mask-relu (feeds conv2) or the ACT m2-relu
            # (feeds conv3): each cross-engine producer gets a full round of
            # slack before its consumer.
            groups = []
            for rep in range(reps):
                for b in range(BPC):
                    for pair in range(2):
                        groups.append((b, pair))
            iters = [(b, pair, py) for (b, pair) in groups for py in range(MS)]
            n = len(iters)
            state = {}   # per-iteration/group tiles carried across stages

            def xrow(t, py):
                # a patch row in (y, px, x) order is just a contiguous
                # 448-column slab of the [c, h*w] tile
                return t[:, 448 * py:448 * (py + 1)]

            def load_group(g):
                b, pair = groups[g]
                xts = [xpool.tile([128, H * W], bf16, tag=f"x{i}",
                                  name=f"xt{g}_{i}") for i in range(4)]
                for i in range(4):
                    c0 = 512 * pair + 128 * i
                    nc.sync.dma_start(
                        xts[i][:],
                        xap[b, c0:c0 + 128].rearrange("c h w -> c (h w)"))
                oas = [opool.tile([128, H * W], bf16, tag=f"oa{i}",
                                  name=f"oa{g}_{i}") for i in range(4)]
                state[("grp", g)] = (xts, oas)

            def stage_a(k):
                b, pair, py = iters[k]
                # prefetch the next group's x tiles ~4.5 rounds before use
                if py == 2 and k // MS + 1 < len(groups):
                    load_group(k // MS + 1)
                xts, oas = state[("grp", k // MS)]
                # ---- conv1: 4 accumulating blockdiag matmuls ----
                p1 = ps1.tile([128, 448], f32, tag="p1", name=f"p1_{k}")
                for j in range(4):
                    nc.tensor.matmul(
                        p1[:],
                        w1t[:, (pair * 4 + j) * 128:(pair * 4 + j + 1) * 128],
                        xrow(xts[j], py),
                        start=(j == 0), stop=(j == 3))
                # ---- fused relu+mask (DVE): m1 = max(p1,0) * mask ----
                # m1 is [row10, patch7, col10] with zero borders; the inner
                # (row 1:9, patch, col 1:9) region collapses to a uniform
                # stride-10 dim of 56 x 8 contiguous cols, so out/in0/in1 are
                # all <=3D as TensorScalarPtr requires, and p1's (y, px, x)
                # stream order matches the (row, patch, col) write order.
                s0 = ((b * 2 + pair) * 7 + py) * 56
                mseg = mkt[:, s0:s0 + 56]
                mbc = mseg.unsqueeze(2).broadcast_to([128, 56, 8])
                p1v = p1[:].rearrange("p (ra c) -> p ra c", c=8)
                m1 = m1s[k % 3]
                m1i = m1[:].rearrange("p (ra c) -> p ra c", c=10)[:, 7:63, 1:9]
                nc.vector.scalar_tensor_tensor(m1i, p1v, 0.0, mbc, Amax, Amul)

            def stage_b(k):
                b, pair, py = iters[k]
                m1 = m1s[k % 3]
                m1v = m1[:]
                m1rc = m1v.rearrange("p (ra c) -> p ra c", c=10)
                APc = type(m1v)
                p2 = ps2.tile([128, 448], f32, tag="p2", name=f"p2_{k}")

                def tap_off(t):
                    return (t // 3) * 70 + (t % 3)

                # fp8 DoubleRow: consecutive taps (2q, 2q+1) become one matmul
                # whose rhs carries the tap pair as a uniform-stride k-subtile
                # dim over the same m1 window; tap 8 runs as a plain matmul.
                for q in range(4):
                    o0, o1 = tap_off(2 * q), tap_off(2 * q + 1)
                    rhs = APc(m1v.tensor, m1v.offset + o0,
                              [list(m1v.ap)[0], [o1 - o0, 2], [10, 56], [1, 8]])
                    lhsT = w2t[:, (pair * 9 + 2 * q) * 128:
                               (pair * 9 + 2 * q + 2) * 128].rearrange(
                                   "p (s m) -> p s m", s=2)
                    nc.tensor.matmul(
                        p2[:], lhsT, rhs, start=(q == 0), stop=False,
                        perf_mode=mybir.MatmulPerfMode.DoubleRow)
                nc.tensor.matmul(
                    p2[:],
                    w2t[:, (pair * 9 + 8) * 128:(pair * 9 + 9) * 128],
                    m1rc[:, 14:14 + 56, 2:10],
                    start=False, stop=True)
                m2 = m2pool.tile([128, 448], bf16, tag="m2", name=f"m2_{k}")
                nc.vector.tensor_scalar_max(m2[:], p2[:], 0.0)
                state[("m2", k)] = m2

            def stage_c(k):
                b, pair, py = iters[k]
                xts, oas = state[("grp", k // MS)]
                m2 = state.pop(("m2", k))
                for mt in range(2):
                    for gi in range(2):
                        blk = (pair * 2 + gi) * 2 + mt
                        ct = 2 * gi + mt
                        p3 = ps3.tile([128, 448], f32, tag="p3", name=f"p3_{k}_{mt}_{gi}")
                        if ct < 2:
                            # residual via PE: preload PSUM with I @ x, then
                            # accumulate conv3 on top; frees the DVE add
                            nc.tensor.matmul(
                                p3[:], idt[:], xrow(xts[ct], py),
                                start=True, stop=False)
                            nc.tensor.matmul(
                                p3[:],
                                w3t[64 * gi:64 * gi + 64, blk * 128:(blk + 1) * 128],
                                m2[64 * gi:64 * gi + 64, :],
                                start=False, stop=True)
                            nc.scalar.activation(xrow(oas[ct], py), p3[:], Relu)
                        else:
                            nc.tensor.matmul(
                                p3[:],
                                w3t[64 * gi:64 * gi + 64, blk * 128:(blk + 1) * 128],
                                m2[64 * gi:64 * gi + 64, :])
                            t3 = t3pool.tile([128, 448], bf16, tag="t3",
                                             name=f"t3_{k}_{ct}")
                            nc.vector.tensor_add(t3[:], p3[:], xrow(xts[ct], py))
                            nc.scalar.activation(xrow(oas[ct], py), t3[:], Relu)
                if py == MS - 1:
                    for i in range(4):
                        c0 = 512 * pair + 128 * i
                        nc.sync.dma_start(
                            oap[b, c0:c0 + 128].rearrange("c h w -> c (h w)"),
                            oas[i][:])
                    del state[("grp", k // MS)]

            load_group(0)
            for k in range(n + 2):
                if k < n:
                    stage_a(k)
                if 1 <= k <= n:
                    stage_b(k - 1)
                if k >= 2:
                    stage_c(k - 2)
    nc.compile()
    return nc


def _get_program():
    if "nc" not in _CACHE:
        _CACHE["nc"] = _build_program()
    return _CACHE["nc"]


def make_in_maps(x, mask, w1, w2, w3):
    x = np.ascontiguousarray(np.asarray(x, np.float32)).astype(_BF16)
    mask = np.asarray(mask, np.float32)
    w1s, w2s, w3s = _pack_weights(np.asarray(w1, np.float32),
                                  np.asarray(w2, np.float32),
                                  np.asarray(w3, np.float32))
    # conv2 runs in fp8e4 (DoubleRow); x16 scales w2 into e4m3's sweet range
    # and the inverse folds into w3, which is linear
    w1s = w1s.astype(_BF16)
    w2s = (w2s * 16.0).astype(_F8)
    w3s = (w3s / 16.0).astype(_BF16)
    ident = np.eye(128, dtype=np.float32).astype(_BF16)
    in_maps = []
    for k in range(NCORES):
        in_maps.append({
            "x": x[BPC * k:BPC * (k + 1)],
            "maskrep": _pack_mask(mask[BPC * k:BPC * (k + 1)]),
            "w1s": w1s, "w2s": w2s, "w3s": w3s, "ident": ident,
        })
    return in_maps


def kernel(x, mask, w1, w2, w3):
    from concourse import bass_utils

    in_maps = make_in_maps(x, mask, w1, w2, w3)
    nc = _get_program()
    res = bass_utils.run_bass_kernel_spmd(nc, in_maps, core_ids=list(range(NCORES)))
    out = np.concatenate([res.results[k]["out"] for k in range(NCORES)], axis=0)
    return out.astype(np.float32)
